# revision 2
# baseline (speedup 1.0000x reference)
"""K-center style kernel: argmax_i min_j ||A_i - B_j|| on 8 NeuronCores.

Strategy:
  - Shard A row-wise over 8 cores (6250 rows each, padded to 6272 = 49*128).
  - Device (per core): m[i] = min_j (||b_j||^2 - 2 a_i.b_j) via TensorE
    matmuls (bf16 inputs, fp32 PSUM accumulate); DVE tensor_tensor(add)
    folds nb into each PSUM chunk (fp32 scratch), then one
    tensor_reduce(min) per 128-row tile.
  - Host: D_approx = sqrt(max(na + m, 0)); select candidate rows within
    DELTA of the max; rescore candidates exactly in float64; return
    (argmax int32, max float32).

The host rescore makes the final answer exact regardless of device matmul
precision; the device pass only needs to be accurate enough that the true
argmax is inside the candidate set (bf16 input rounding gives |D err| on
the order of 1e-2; DELTA is set far above that).
"""

import numpy as np
import ml_dtypes

N_CORES = 8
N_TOTAL = 50000
M_B = 5000
D_FEAT = 512
N_PER_CORE = N_TOTAL // N_CORES          # 6250
ROW_TILES = 49                            # ceil(6250/128)
N_PAD = ROW_TILES * 128                   # 6272
K_TILES = 4                               # 512 / 128
N_CHUNK = 500                             # free-dim chunk (<=512 fp32 PSUM bank)
N_CHUNKS = M_B // N_CHUNK                 # 10

DELTA = 0.1  # candidate slack in D units; device bf16 |D| error << this

_compiled = None


def _build_program():
    import concourse.tile as tile
    import concourse.mybir as mybir
    from concourse import bacc

    nc = bacc.Bacc("TRN2", target_bir_lowering=False, debug=False)
    atb = nc.dram_tensor(
        "ATB", [ROW_TILES, 128, 512], mybir.dt.bfloat16, kind="ExternalInput"
    ).ap()
    btb = nc.dram_tensor(
        "BTB", [128, K_TILES * M_B], mybir.dt.bfloat16, kind="ExternalInput"
    ).ap()
    nbb = nc.dram_tensor(
        "NBB", [128, M_B], mybir.dt.float32, kind="ExternalInput"
    ).ap()
    mout = nc.dram_tensor(
        "M", [128, ROW_TILES], mybir.dt.float32, kind="ExternalOutput"
    ).ap()

    fp32 = mybir.dt.float32
    bf16 = mybir.dt.bfloat16
    add = mybir.AluOpType.add
    amin = mybir.AluOpType.min

    with tile.TileContext(nc) as tc:
        with (
            tc.tile_pool(name="const", bufs=1) as cpool,
            tc.tile_pool(name="arows", bufs=4) as apool,
            tc.tile_pool(name="psum", bufs=8, space="PSUM") as pspool,
            tc.tile_pool(name="scr", bufs=2) as spool,
            tc.tile_pool(name="mout", bufs=1) as mpool,
        ):
            bt_sb = cpool.tile([128, K_TILES * M_B], bf16)
            for k in range(K_TILES):
                nc.sync.dma_start(
                    out=bt_sb[:, k * M_B : (k + 1) * M_B],
                    in_=btb[:, k * M_B : (k + 1) * M_B],
                )
            nb_sb = cpool.tile([128, M_B], fp32)
            nc.sync.dma_start(out=nb_sb[:], in_=nbb[:])
            m_sb = mpool.tile([128, ROW_TILES], fp32)

            for it in range(ROW_TILES):
                a_sb = apool.tile([128, 512], bf16)
                nc.sync.dma_start(out=a_sb[:], in_=atb[it])
                scr = spool.tile([128, M_B], fp32)
                for n in range(N_CHUNKS):
                    ps = pspool.tile([128, N_CHUNK], fp32)
                    for k in range(K_TILES):
                        nc.tensor.matmul(
                            ps[:],
                            lhsT=a_sb[:, k * 128 : (k + 1) * 128],
                            rhs=bt_sb[:, k * M_B + n * N_CHUNK : k * M_B + (n + 1) * N_CHUNK],
                            start=(k == 0),
                            stop=(k == K_TILES - 1),
                        )
                    nc.vector.tensor_tensor(
                        out=scr[:, n * N_CHUNK : (n + 1) * N_CHUNK],
                        in0=ps[:],
                        in1=nb_sb[:, n * N_CHUNK : (n + 1) * N_CHUNK],
                        op=add,
                    )
                nc.vector.tensor_reduce(
                    out=m_sb[:, it : it + 1],
                    in_=scr[:],
                    axis=mybir.AxisListType.X,
                    op=amin,
                )
            nc.sync.dma_start(out=mout[:], in_=m_sb[:])
    nc.compile()
    return nc


def _prep_inputs(A, B):
    bf16 = ml_dtypes.bfloat16
    # ATB: per-core row-tile blocks [core, 49, 128p(feat%128), 4k*128i] of -2A
    Apad = np.zeros((N_CORES, N_PAD, D_FEAT), np.float32)
    Apad[:, :N_PER_CORE, :] = (-2.0 * A.astype(np.float32)).reshape(
        N_CORES, N_PER_CORE, D_FEAT
    )
    atb = np.ascontiguousarray(
        Apad.reshape(N_CORES, ROW_TILES, 128, K_TILES, 128).transpose(0, 1, 4, 3, 2)
    ).reshape(N_CORES, ROW_TILES, 128, 512).astype(bf16)

    # BTB: [128p, 4k, 5000j] = B[j, k*128+p]
    btb = np.ascontiguousarray(
        B.astype(np.float32).reshape(M_B, K_TILES, 128).transpose(2, 1, 0)
    ).reshape(128, K_TILES * M_B).astype(bf16)

    nb32 = (B.astype(np.float32) ** 2).sum(axis=1)
    nbb = np.ascontiguousarray(np.broadcast_to(nb32[None, :], (128, M_B))).astype(
        np.float32
    )
    return atb, btb, nbb


def _exact_rescore(A, B, cand):
    A64 = A[cand].astype(np.float64)
    B64 = B.astype(np.float64)
    na = (A64 * A64).sum(axis=1)[:, None]
    nb = (B64 * B64).sum(axis=1)[None, :]
    sq = na - 2.0 * (A64 @ B64.T) + nb
    d = np.sqrt(np.maximum(sq, 0.0))
    return d.min(axis=1)


def kernel(A, B, _trace=False):
    from concourse.bass_utils import run_bass_kernel_spmd

    global _compiled
    if _compiled is None:
        _compiled = _build_program()
    nc = _compiled

    A = np.asarray(A, np.float32)
    B = np.asarray(B, np.float32)
    atb, btb, nbb = _prep_inputs(A, B)
    in_maps = [
        {"ATB": atb[c], "BTB": btb, "NBB": nbb} for c in range(N_CORES)
    ]
    res = run_bass_kernel_spmd(nc, in_maps, list(range(N_CORES)), trace=_trace)

    # Gather per-core m and undo the [128, 49] (p, it) layout -> row it*128+p
    m = np.concatenate(
        [res.results[c]["M"].T.reshape(-1)[:N_PER_CORE] for c in range(N_CORES)]
    )
    na = (A.astype(np.float64) ** 2).sum(axis=1)
    d_approx = np.sqrt(np.maximum(na + m, 0.0))
    v = d_approx.max()
    cand = np.where(d_approx >= v - DELTA)[0]
    d_exact = _exact_rescore(A, B, cand)
    w = int(np.argmax(d_exact))
    idx = int(cand[w])
    val = float(d_exact[w])
    out = (np.array(idx, dtype=np.int32), np.array(val, dtype=np.float32))
    if _trace:
        return out, res
    return out


# revision 4
# speedup vs baseline: 1.1552x; 1.1552x over previous
"""K-center style kernel: argmax_i min_j ||A_i - B_j|| on 8 NeuronCores.

Strategy:
  - Shard A row-wise over 8 cores (6250 rows each, padded to 6272 = 49*128).
  - Host: pad B to 5120 columns (copies of one real column), sort by
    nb = ||b||^2; group into runs of G=16 sorted columns with per-group
    nb midpoint.
  - Device (per core): matmuls (bf16, fp32 PSUM) produce -2 a_i.b_j in
    4-bank PSUM tiles (chunks of 512, bank aligned); one grouped DVE
    tensor_reduce(min) per PSUM tile gives per-group minima of -2p;
    per row-tile a tiny add(nb_mid) + min-reduce yields
    m[i] ~= min_j (nb_j - 2 a_i.b_j).
  - Host: D_approx = sqrt(max(na + m, 0)); select candidate rows within
    DELTA of the max; rescore candidates exactly in float64; return
    (argmax int32, max float32).

The host rescore makes the final answer exact regardless of device
precision; the device pass only needs the true argmax inside the
candidate set. Device error sources: bf16 input rounding (|D err| ~1e-2)
+ nb grouping (~1e-2). DELTA = 0.1 is far above both.
"""

import numpy as np
import ml_dtypes

N_CORES = 8
N_TOTAL = 50000
M_B = 5000
M_PAD = 5120                              # padded B columns (10 * 512)
D_FEAT = 512
N_PER_CORE = N_TOTAL // N_CORES          # 6250
ROW_TILES = 49                            # ceil(6250/128)
N_PAD = ROW_TILES * 128                   # 6272
K_TILES = 4                               # 512 / 128
N_CHUNK = 512                             # matmul free dim = one fp32 PSUM bank
GRP = 16                                  # B columns per min-group (sorted by nb)

DELTA = 0.1  # candidate slack in D units

_compiled = None


def build_program(row_tiles=ROW_TILES, m_b=M_PAD, k_tiles=K_TILES, n_chunk=N_CHUNK, grp=GRP):
    import concourse.tile as tile
    import concourse.mybir as mybir
    from concourse import bacc

    n_chunks = m_b // n_chunk
    n_groups = m_b // grp
    gpc = n_chunk // grp                 # groups per chunk
    assert m_b % n_chunk == 0 and n_chunk % grp == 0

    nc = bacc.Bacc("TRN2", target_bir_lowering=False, debug=False)
    atb = nc.dram_tensor(
        "ATB", [row_tiles, 128, k_tiles * 128], mybir.dt.bfloat16, kind="ExternalInput"
    ).ap()
    btb = nc.dram_tensor(
        "BTB", [128, k_tiles * m_b], mybir.dt.bfloat16, kind="ExternalInput"
    ).ap()
    nbg = nc.dram_tensor(
        "NBG", [128, n_groups], mybir.dt.float32, kind="ExternalInput"
    ).ap()
    mout = nc.dram_tensor(
        "M", [128, row_tiles], mybir.dt.float32, kind="ExternalOutput"
    ).ap()

    fp32 = mybir.dt.float32
    bf16 = mybir.dt.bfloat16
    add = mybir.AluOpType.add
    amin = mybir.AluOpType.min
    X = mybir.AxisListType.X

    # chunk groups of up to 4 chunks -> one 4-bank PSUM tile each
    psgroups = []
    c = 0
    while c < n_chunks:
        w = min(4, n_chunks - c)
        psgroups.append((c, w))
        c += w

    with tile.TileContext(nc) as tc:
        with (
            tc.tile_pool(name="const", bufs=1) as cpool,
            tc.tile_pool(name="arows", bufs=4) as apool,
            tc.tile_pool(name="psum", bufs=2, space="PSUM") as pspool,
            tc.tile_pool(name="gm", bufs=3) as gmpool,
            tc.tile_pool(name="mout", bufs=1) as mpool,
        ):
            bt_sb = cpool.tile([128, k_tiles * m_b], bf16)
            for k in range(k_tiles):
                nc.sync.dma_start(
                    out=bt_sb[:, k * m_b : (k + 1) * m_b],
                    in_=btb[:, k * m_b : (k + 1) * m_b],
                )
            nbg_sb = cpool.tile([128, n_groups], fp32)
            nc.sync.dma_start(out=nbg_sb[:], in_=nbg[:])
            m_sb = mpool.tile([128, row_tiles], fp32)

            for it in range(row_tiles):
                a_sb = apool.tile([128, k_tiles * 128], bf16)
                nc.sync.dma_start(out=a_sb[:], in_=atb[it])
                gm_sb = gmpool.tile([128, n_groups], fp32)
                for c0, w in psgroups:
                    ps = pspool.tile([128, 4 * n_chunk], fp32)
                    for nl in range(w):
                        n = c0 + nl
                        for k in range(k_tiles):
                            nc.tensor.matmul(
                                ps[:, nl * n_chunk : (nl + 1) * n_chunk],
                                lhsT=a_sb[:, k * 128 : (k + 1) * 128],
                                rhs=bt_sb[:, k * m_b + n * n_chunk : k * m_b + (n + 1) * n_chunk],
                                start=(k == 0),
                                stop=(k == k_tiles - 1),
                            )
                    nc.vector.tensor_reduce(
                        out=gm_sb[:, c0 * gpc : (c0 + w) * gpc],
                        in_=ps[:, : w * n_chunk].rearrange("p (a b) -> p a b", b=grp),
                        axis=X,
                        op=amin,
                    )
                s_sb = gmpool.tile([128, n_groups], fp32, tag="s")
                nc.vector.tensor_tensor(
                    out=s_sb[:], in0=gm_sb[:], in1=nbg_sb[:], op=add
                )
                nc.vector.tensor_reduce(
                    out=m_sb[:, it : it + 1], in_=s_sb[:], axis=X, op=amin
                )
            nc.sync.dma_start(out=mout[:], in_=m_sb[:])
    nc.compile()
    return nc


def prep_inputs(A, B):
    """A: [N, 512] f32 (full), B: [M, 512] f32. Returns atb, btb, nbg."""
    bf16 = ml_dtypes.bfloat16
    B32 = B.astype(np.float32)
    nb32 = (B32**2).sum(axis=1)
    # pad B with copies of column 0 (distance contributions duplicate, min unchanged)
    Bp = np.concatenate([B32, np.broadcast_to(B32[0:1], (M_PAD - M_B, D_FEAT))], axis=0)
    nbp = np.concatenate([nb32, np.broadcast_to(nb32[0:1], (M_PAD - M_B,))])
    order = np.argsort(nbp, kind="stable")
    Bs = Bp[order]
    nbs = nbp[order]

    # ATB: per-core row-tile blocks [core, 49, 128p(feat%128), 4k*128i] of -2A
    Apad = np.zeros((N_CORES, N_PAD, D_FEAT), np.float32)
    Apad[:, :N_PER_CORE, :] = (-2.0 * A.astype(np.float32)).reshape(
        N_CORES, N_PER_CORE, D_FEAT
    )
    atb = np.ascontiguousarray(
        Apad.reshape(N_CORES, ROW_TILES, 128, K_TILES, 128).transpose(0, 1, 4, 3, 2)
    ).reshape(N_CORES, ROW_TILES, 128, 512).astype(bf16)

    # BTB: [128p, 4k, 5120j] = Bs[j, k*128+p]
    btb = np.ascontiguousarray(
        Bs.reshape(M_PAD, K_TILES, 128).transpose(2, 1, 0)
    ).reshape(128, K_TILES * M_PAD).astype(bf16)

    # per-group nb midpoint
    g = nbs.reshape(M_PAD // GRP, GRP)
    nb_mid = ((g.min(axis=1) + g.max(axis=1)) * 0.5).astype(np.float32)
    nbg = np.ascontiguousarray(
        np.broadcast_to(nb_mid[None, :], (128, M_PAD // GRP))
    ).astype(np.float32)
    return atb, btb, nbg


def _exact_rescore(A, B, cand):
    A64 = A[cand].astype(np.float64)
    B64 = B.astype(np.float64)
    na = (A64 * A64).sum(axis=1)[:, None]
    nb = (B64 * B64).sum(axis=1)[None, :]
    sq = na - 2.0 * (A64 @ B64.T) + nb
    d = np.sqrt(np.maximum(sq, 0.0))
    return d.min(axis=1)


def kernel(A, B, _trace=False):
    from concourse.bass_utils import run_bass_kernel_spmd

    global _compiled
    if _compiled is None:
        _compiled = build_program()
    nc = _compiled

    A = np.asarray(A, np.float32)
    B = np.asarray(B, np.float32)
    atb, btb, nbg = prep_inputs(A, B)
    in_maps = [{"ATB": atb[c], "BTB": btb, "NBG": nbg} for c in range(N_CORES)]
    res = run_bass_kernel_spmd(nc, in_maps, list(range(N_CORES)), trace=_trace)

    # Gather per-core m and undo the [128, 49] (p, it) layout -> row it*128+p
    m = np.concatenate(
        [res.results[c]["M"].T.reshape(-1)[:N_PER_CORE] for c in range(N_CORES)]
    )
    na = (A.astype(np.float64) ** 2).sum(axis=1)
    d_approx = np.sqrt(np.maximum(na + m, 0.0))
    v = d_approx.max()
    cand = np.where(d_approx >= v - DELTA)[0]
    d_exact = _exact_rescore(A, B, cand)
    w = int(np.argmax(d_exact))
    idx = int(cand[w])
    val = float(d_exact[w])
    out = (np.array(idx, dtype=np.int32), np.array(val, dtype=np.float32))
    if _trace:
        return out, res
    return out


# revision 5
# speedup vs baseline: 1.1654x; 1.0088x over previous
"""K-center style kernel: argmax_i min_j ||A_i - B_j|| on 8 NeuronCores.

Strategy:
  - Shard A row-wise over 8 cores (6250 rows each, padded to 6272 = 49*128).
  - Host: pad B to 5120 columns (copies of one real column), sort by
    nb = ||b||^2; group into runs of G=16 sorted columns with per-group
    nb midpoint.
  - Device (per core): matmuls (bf16, fp32 PSUM) produce -2 a_i.b_j in
    4-bank PSUM tiles (chunks of 512, bank aligned); one grouped DVE
    tensor_reduce(min) per PSUM tile gives per-group minima of -2p;
    per row-tile a tiny add(nb_mid) + min-reduce yields
    m[i] ~= min_j (nb_j - 2 a_i.b_j).
  - Host: D_approx = sqrt(max(na + m, 0)); select candidate rows within
    DELTA of the max; rescore candidates exactly in float64; return
    (argmax int32, max float32).

The host rescore makes the final answer exact regardless of device
precision; the device pass only needs the true argmax inside the
candidate set. Device error sources: bf16 input rounding (|D err| ~1e-2)
+ nb grouping (~1e-2). DELTA = 0.1 is far above both.
"""

import numpy as np
import ml_dtypes

N_CORES = 8
N_TOTAL = 50000
M_B = 5000
M_PAD = 5120                              # padded B columns (10 * 512)
D_FEAT = 512
N_PER_CORE = N_TOTAL // N_CORES          # 6250
ROW_TILES = 49                            # ceil(6250/128)
N_PAD = ROW_TILES * 128                   # 6272
K_TILES = 4                               # 512 / 128
N_CHUNK = 512                             # matmul free dim = one fp32 PSUM bank
GRP = 32                                  # B columns per min-group (sorted by nb)

DELTA = 0.15  # candidate slack in D units

_compiled = None


def build_program(row_tiles=ROW_TILES, m_b=M_PAD, k_tiles=K_TILES, n_chunk=N_CHUNK, grp=GRP):
    import concourse.tile as tile
    import concourse.mybir as mybir
    from concourse import bacc

    n_chunks = m_b // n_chunk
    n_groups = m_b // grp
    gpc = n_chunk // grp                 # groups per chunk
    assert m_b % n_chunk == 0 and n_chunk % grp == 0

    nc = bacc.Bacc("TRN2", target_bir_lowering=False, debug=False)
    atb = nc.dram_tensor(
        "ATB", [row_tiles, 128, k_tiles * 128], mybir.dt.bfloat16, kind="ExternalInput"
    ).ap()
    btb = nc.dram_tensor(
        "BTB", [128, k_tiles * m_b], mybir.dt.bfloat16, kind="ExternalInput"
    ).ap()
    nbg = nc.dram_tensor(
        "NBG", [128, n_groups], mybir.dt.float32, kind="ExternalInput"
    ).ap()
    mout = nc.dram_tensor(
        "M", [128, row_tiles], mybir.dt.float32, kind="ExternalOutput"
    ).ap()

    fp32 = mybir.dt.float32
    bf16 = mybir.dt.bfloat16
    add = mybir.AluOpType.add
    amin = mybir.AluOpType.min
    X = mybir.AxisListType.X

    # chunk groups -> one PSUM tile each; first group small so the DVE
    # drain of this row-tile starts early and finishes with the PE stream
    psgroups = []
    c = 0
    first = True
    while c < n_chunks:
        w = min(2 if first else 4, n_chunks - c)
        first = False
        psgroups.append((c, w))
        c += w

    with tile.TileContext(nc) as tc:
        with (
            tc.tile_pool(name="const", bufs=1) as cpool,
            tc.tile_pool(name="arows", bufs=4) as apool,
            tc.tile_pool(name="psum", bufs=2, space="PSUM") as pspool,
            tc.tile_pool(name="gm", bufs=3) as gmpool,
            tc.tile_pool(name="mout", bufs=1) as mpool,
        ):
            bt_sb = cpool.tile([128, k_tiles * m_b], bf16)
            for c0, w in psgroups:
                for k in range(k_tiles):
                    lo = k * m_b + c0 * n_chunk
                    hi = k * m_b + (c0 + w) * n_chunk
                    nc.sync.dma_start(out=bt_sb[:, lo:hi], in_=btb[:, lo:hi])
            nbg_sb = cpool.tile([128, n_groups], fp32)
            nc.sync.dma_start(out=nbg_sb[:], in_=nbg[:])
            m_sb = mpool.tile([128, row_tiles], fp32)

            for it in range(row_tiles):
                a_sb = apool.tile([128, k_tiles * 128], bf16)
                nc.sync.dma_start(out=a_sb[:], in_=atb[it])
                gm_sb = gmpool.tile([128, n_groups], fp32)
                for c0, w in psgroups:
                    ps = pspool.tile([128, 4 * n_chunk], fp32)
                    for nl in range(w):
                        n = c0 + nl
                        for k in range(k_tiles):
                            nc.tensor.matmul(
                                ps[:, nl * n_chunk : (nl + 1) * n_chunk],
                                lhsT=a_sb[:, k * 128 : (k + 1) * 128],
                                rhs=bt_sb[:, k * m_b + n * n_chunk : k * m_b + (n + 1) * n_chunk],
                                start=(k == 0),
                                stop=(k == k_tiles - 1),
                            )
                    nc.vector.tensor_reduce(
                        out=gm_sb[:, c0 * gpc : (c0 + w) * gpc],
                        in_=ps[:, : w * n_chunk].rearrange("p (a b) -> p a b", b=grp),
                        axis=X,
                        op=amin,
                    )
                s_sb = gmpool.tile([128, n_groups], fp32, tag="s")
                nc.vector.tensor_tensor(
                    out=s_sb[:], in0=gm_sb[:], in1=nbg_sb[:], op=add
                )
                nc.vector.tensor_reduce(
                    out=m_sb[:, it : it + 1], in_=s_sb[:], axis=X, op=amin
                )
            nc.sync.dma_start(out=mout[:], in_=m_sb[:])
    nc.compile()
    return nc


def prep_inputs(A, B):
    """A: [N, 512] f32 (full), B: [M, 512] f32. Returns atb, btb, nbg."""
    bf16 = ml_dtypes.bfloat16
    B32 = B.astype(np.float32)
    nb32 = (B32**2).sum(axis=1)
    # pad B with copies of column 0 (distance contributions duplicate, min unchanged)
    Bp = np.concatenate([B32, np.broadcast_to(B32[0:1], (M_PAD - M_B, D_FEAT))], axis=0)
    nbp = np.concatenate([nb32, np.broadcast_to(nb32[0:1], (M_PAD - M_B,))])
    order = np.argsort(nbp, kind="stable")
    Bs = Bp[order]
    nbs = nbp[order]

    # ATB: per-core row-tile blocks [core, 49, 128p(feat%128), 4k*128i] of -2A
    Apad = np.zeros((N_CORES, N_PAD, D_FEAT), np.float32)
    Apad[:, :N_PER_CORE, :] = (-2.0 * A.astype(np.float32)).reshape(
        N_CORES, N_PER_CORE, D_FEAT
    )
    atb = np.ascontiguousarray(
        Apad.reshape(N_CORES, ROW_TILES, 128, K_TILES, 128).transpose(0, 1, 4, 3, 2)
    ).reshape(N_CORES, ROW_TILES, 128, 512).astype(bf16)

    # BTB: [128p, 4k, 5120j] = Bs[j, k*128+p]
    btb = np.ascontiguousarray(
        Bs.reshape(M_PAD, K_TILES, 128).transpose(2, 1, 0)
    ).reshape(128, K_TILES * M_PAD).astype(bf16)

    # per-group nb midpoint
    g = nbs.reshape(M_PAD // GRP, GRP)
    nb_mid = ((g.min(axis=1) + g.max(axis=1)) * 0.5).astype(np.float32)
    nbg = np.ascontiguousarray(
        np.broadcast_to(nb_mid[None, :], (128, M_PAD // GRP))
    ).astype(np.float32)
    return atb, btb, nbg


def _exact_rescore(A, B, cand):
    A64 = A[cand].astype(np.float64)
    B64 = B.astype(np.float64)
    na = (A64 * A64).sum(axis=1)[:, None]
    nb = (B64 * B64).sum(axis=1)[None, :]
    sq = na - 2.0 * (A64 @ B64.T) + nb
    d = np.sqrt(np.maximum(sq, 0.0))
    return d.min(axis=1)


def kernel(A, B, _trace=False):
    from concourse.bass_utils import run_bass_kernel_spmd

    global _compiled
    if _compiled is None:
        _compiled = build_program()
    nc = _compiled

    A = np.asarray(A, np.float32)
    B = np.asarray(B, np.float32)
    atb, btb, nbg = prep_inputs(A, B)
    in_maps = [{"ATB": atb[c], "BTB": btb, "NBG": nbg} for c in range(N_CORES)]
    res = run_bass_kernel_spmd(nc, in_maps, list(range(N_CORES)), trace=_trace)

    # Gather per-core m and undo the [128, 49] (p, it) layout -> row it*128+p
    m = np.concatenate(
        [res.results[c]["M"].T.reshape(-1)[:N_PER_CORE] for c in range(N_CORES)]
    )
    na = (A.astype(np.float64) ** 2).sum(axis=1)
    d_approx = np.sqrt(np.maximum(na + m, 0.0))
    v = d_approx.max()
    cand = np.where(d_approx >= v - DELTA)[0]
    d_exact = _exact_rescore(A, B, cand)
    w = int(np.argmax(d_exact))
    idx = int(cand[w])
    val = float(d_exact[w])
    out = (np.array(idx, dtype=np.int32), np.array(val, dtype=np.float32))
    if _trace:
        return out, res
    return out


# revision 8
# speedup vs baseline: 1.2939x; 1.1102x over previous
"""K-center style kernel: argmax_i min_j ||A_i - B_j|| on 8 NeuronCores.

Strategy:
  - Shard A row-wise over 8 cores (6250 rows each, padded to 6272 = 49*128).
  - Host: pad B to 5120 columns (copies of one real column), sort by
    nb = ||b||^2; group into runs of G=16 sorted columns with per-group
    nb midpoint.
  - Device (per core): matmuls (bf16, fp32 PSUM) produce -2 a_i.b_j in
    4-bank PSUM tiles (chunks of 512, bank aligned); one grouped DVE
    tensor_reduce(min) per PSUM tile gives per-group minima of -2p;
    per row-tile a tiny add(nb_mid) + min-reduce yields
    m[i] ~= min_j (nb_j - 2 a_i.b_j).
  - Host: D_approx = sqrt(max(na + m, 0)); select candidate rows within
    DELTA of the max; rescore candidates exactly in float64; return
    (argmax int32, max float32).

The host rescore makes the final answer exact regardless of device
precision; the device pass only needs the true argmax inside the
candidate set. Device error sources: bf16 input rounding (|D err| ~1e-2)
+ nb grouping (~1e-2). DELTA = 0.1 is far above both.
"""

import numpy as np
import ml_dtypes

N_CORES = 8
N_TOTAL = 50000
M_B = 5000
M_PAD = 5120                              # padded B columns (10 * 512)
D_FEAT = 512
N_PER_CORE = N_TOTAL // N_CORES          # 6250
ROW_TILES = 49                            # ceil(6250/128)
N_PAD = ROW_TILES * 128                   # 6272
K_TILES = 4                               # 512 / 128
N_CHUNK = 512                             # matmul free dim = one fp32 PSUM bank
GRP = 32                                  # B columns per min-group (sorted by nb)

DELTA = 0.15  # candidate slack in D units

_compiled = None


def build_program(row_tiles=ROW_TILES, m_b=M_PAD, k_tiles=K_TILES, n_chunk=N_CHUNK, grp=GRP):
    import concourse.tile as tile
    import concourse.mybir as mybir
    from concourse import bacc

    n_chunks = m_b // n_chunk
    n_groups = m_b // grp
    gpc = n_chunk // grp                 # groups per chunk
    assert m_b % n_chunk == 0 and n_chunk % grp == 0

    nc = bacc.Bacc("TRN2", target_bir_lowering=False, debug=False)
    atb = nc.dram_tensor(
        "ATB", [row_tiles, 128, k_tiles * 128], mybir.dt.bfloat16, kind="ExternalInput"
    ).ap()
    btb = nc.dram_tensor(
        "BTB", [128, k_tiles * m_b], mybir.dt.bfloat16, kind="ExternalInput"
    ).ap()
    nbg = nc.dram_tensor(
        "NBG", [128, n_groups], mybir.dt.float32, kind="ExternalInput"
    ).ap()
    mout = nc.dram_tensor(
        "M", [128, row_tiles], mybir.dt.float32, kind="ExternalOutput"
    ).ap()

    fp32 = mybir.dt.float32
    bf16 = mybir.dt.bfloat16
    add = mybir.AluOpType.add
    amin = mybir.AluOpType.min
    X = mybir.AxisListType.X

    # chunk groups -> one PSUM tile each; first group small so the DVE
    # drain of this row-tile starts early and finishes with the PE stream
    psgroups = []
    c = 0
    first = True
    while c < n_chunks:
        w = min(2 if first else 4, n_chunks - c)
        first = False
        psgroups.append((c, w))
        c += w

    with tile.TileContext(nc) as tc:
        with (
            tc.tile_pool(name="const", bufs=1) as cpool,
            tc.tile_pool(name="psum", bufs=2, space="PSUM") as pspool,
            tc.tile_pool(name="gm", bufs=row_tiles) as gmpool,
            tc.tile_pool(name="sfin", bufs=3) as spool,
            tc.tile_pool(name="mout", bufs=1) as mpool,
        ):
            # All of A^T resident: [128, row_tiles*512] bf16, one DMA per
            # row-tile on the sync queue (first matmul only needs piece 0).
            a_all = cpool.tile([128, row_tiles * k_tiles * 128], bf16)
            for it in range(row_tiles):
                nc.sync.dma_start(
                    out=a_all[:, it * 512 : (it + 1) * 512], in_=atb[it]
                )
            # B^T + nb on the scalar queue so they don't serialize behind ATB.
            bt_sb = cpool.tile([128, k_tiles * m_b], bf16)
            for c0, w in psgroups:
                for k in range(k_tiles):
                    lo = k * m_b + c0 * n_chunk
                    hi = k * m_b + (c0 + w) * n_chunk
                    nc.scalar.dma_start(out=bt_sb[:, lo:hi], in_=btb[:, lo:hi])
            nbg_sb = cpool.tile([128, n_groups], fp32)
            nc.scalar.dma_start(out=nbg_sb[:], in_=nbg[:])
            m_sb = mpool.tile([128, row_tiles], fp32)

            gm_tiles = [gmpool.tile([128, n_groups], fp32, tag="gm", name=f"gm{i}") for i in range(row_tiles)]
            last_c0 = psgroups[-1][0]
            for c0, w in psgroups:
                for it in range(row_tiles):
                    ps = pspool.tile([128, 4 * n_chunk], fp32)
                    for nl in range(w):
                        n = c0 + nl
                        for k in range(k_tiles):
                            nc.tensor.matmul(
                                ps[:, nl * n_chunk : (nl + 1) * n_chunk],
                                lhsT=a_all[:, it * 512 + k * 128 : it * 512 + (k + 1) * 128],
                                rhs=bt_sb[:, k * m_b + n * n_chunk : k * m_b + (n + 1) * n_chunk],
                                start=(k == 0),
                                stop=(k == k_tiles - 1),
                            )
                    nc.vector.tensor_reduce(
                        out=gm_tiles[it][:, c0 * gpc : (c0 + w) * gpc],
                        in_=ps[:, : w * n_chunk].rearrange("p (a b) -> p a b", b=grp),
                        axis=X,
                        op=amin,
                    )
                    if c0 == last_c0:
                        s_sb = spool.tile([128, n_groups], fp32)
                        nc.vector.tensor_tensor(
                            out=s_sb[:], in0=gm_tiles[it][:], in1=nbg_sb[:], op=add
                        )
                        nc.vector.tensor_reduce(
                            out=m_sb[:, it : it + 1], in_=s_sb[:], axis=X, op=amin
                        )
            nc.sync.dma_start(out=mout[:], in_=m_sb[:])
    nc.compile()
    return nc


def prep_inputs(A, B):
    """A: [N, 512] f32 (full), B: [M, 512] f32. Returns atb, btb, nbg."""
    bf16 = ml_dtypes.bfloat16
    B32 = B.astype(np.float32)
    nb32 = (B32**2).sum(axis=1)
    # pad B with copies of column 0 (distance contributions duplicate, min unchanged)
    Bp = np.concatenate([B32, np.broadcast_to(B32[0:1], (M_PAD - M_B, D_FEAT))], axis=0)
    nbp = np.concatenate([nb32, np.broadcast_to(nb32[0:1], (M_PAD - M_B,))])
    order = np.argsort(nbp, kind="stable")
    Bs = Bp[order]
    nbs = nbp[order]

    # ATB: per-core row-tile blocks [core, 49, 128p(feat%128), 4k*128i] of -2A
    Apad = np.zeros((N_CORES, N_PAD, D_FEAT), np.float32)
    Apad[:, :N_PER_CORE, :] = (-2.0 * A.astype(np.float32)).reshape(
        N_CORES, N_PER_CORE, D_FEAT
    )
    atb = np.ascontiguousarray(
        Apad.reshape(N_CORES, ROW_TILES, 128, K_TILES, 128).transpose(0, 1, 4, 3, 2)
    ).reshape(N_CORES, ROW_TILES, 128, 512).astype(bf16)

    # BTB: [128p, 4k, 5120j] = Bs[j, k*128+p]
    btb = np.ascontiguousarray(
        Bs.reshape(M_PAD, K_TILES, 128).transpose(2, 1, 0)
    ).reshape(128, K_TILES * M_PAD).astype(bf16)

    # per-group nb midpoint
    g = nbs.reshape(M_PAD // GRP, GRP)
    nb_mid = ((g.min(axis=1) + g.max(axis=1)) * 0.5).astype(np.float32)
    nbg = np.ascontiguousarray(
        np.broadcast_to(nb_mid[None, :], (128, M_PAD // GRP))
    ).astype(np.float32)
    return atb, btb, nbg


def _exact_rescore(A, B, cand):
    A64 = A[cand].astype(np.float64)
    B64 = B.astype(np.float64)
    na = (A64 * A64).sum(axis=1)[:, None]
    nb = (B64 * B64).sum(axis=1)[None, :]
    sq = na - 2.0 * (A64 @ B64.T) + nb
    d = np.sqrt(np.maximum(sq, 0.0))
    return d.min(axis=1)


def kernel(A, B, _trace=False):
    from concourse.bass_utils import run_bass_kernel_spmd

    global _compiled
    if _compiled is None:
        _compiled = build_program()
    nc = _compiled

    A = np.asarray(A, np.float32)
    B = np.asarray(B, np.float32)
    atb, btb, nbg = prep_inputs(A, B)
    in_maps = [{"ATB": atb[c], "BTB": btb, "NBG": nbg} for c in range(N_CORES)]
    res = run_bass_kernel_spmd(nc, in_maps, list(range(N_CORES)), trace=_trace)

    # Gather per-core m and undo the [128, 49] (p, it) layout -> row it*128+p
    m = np.concatenate(
        [res.results[c]["M"].T.reshape(-1)[:N_PER_CORE] for c in range(N_CORES)]
    )
    na = (A.astype(np.float64) ** 2).sum(axis=1)
    d_approx = np.sqrt(np.maximum(na + m, 0.0))
    v = d_approx.max()
    cand = np.where(d_approx >= v - DELTA)[0]
    d_exact = _exact_rescore(A, B, cand)
    w = int(np.argmax(d_exact))
    idx = int(cand[w])
    val = float(d_exact[w])
    out = (np.array(idx, dtype=np.int32), np.array(val, dtype=np.float32))
    if _trace:
        return out, res
    return out


# revision 9
# speedup vs baseline: 1.8221x; 1.4082x over previous
"""K-center style kernel: argmax_i min_j ||A_i - B_j|| on 8 NeuronCores.

Strategy:
  - Shard A row-wise over 8 cores (6250 rows each, padded to 6272 = 49*128).
  - Host: pad B to 5120 columns (copies of one real column), sort by
    nb = ||b||^2; group into runs of G=16 sorted columns with per-group
    nb midpoint.
  - Device (per core): matmuls (bf16, fp32 PSUM) produce -2 a_i.b_j in
    4-bank PSUM tiles (chunks of 512, bank aligned); one grouped DVE
    tensor_reduce(min) per PSUM tile gives per-group minima of -2p;
    per row-tile a tiny add(nb_mid) + min-reduce yields
    m[i] ~= min_j (nb_j - 2 a_i.b_j).
  - Host: D_approx = sqrt(max(na + m, 0)); select candidate rows within
    DELTA of the max; rescore candidates exactly in float64; return
    (argmax int32, max float32).

The host rescore makes the final answer exact regardless of device
precision; the device pass only needs the true argmax inside the
candidate set. Device error sources: bf16 input rounding (|D err| ~1e-2)
+ nb grouping (~1e-2). DELTA = 0.1 is far above both.
"""

import numpy as np
import ml_dtypes

N_CORES = 8
N_TOTAL = 50000
M_B = 5000
M_PAD = 5120                              # padded B columns (10 * 512)
D_FEAT = 512
N_PER_CORE = N_TOTAL // N_CORES          # 6250
ROW_TILES = 49                            # ceil(6250/128)
N_PAD = ROW_TILES * 128                   # 6272
K_TILES = 2                               # 512 / 256 (DoubleRow: 256 K per pass)
N_CHUNK = 512                             # matmul free dim = one fp32 PSUM bank
GRP = 32                                  # B columns per min-group (sorted by nb)

DELTA = 1.0  # candidate slack in D units (covers fp8 e4m3 + grouping error)

_compiled = None


def build_program(row_tiles=ROW_TILES, m_b=M_PAD, k_tiles=K_TILES, n_chunk=N_CHUNK, grp=GRP):
    import concourse.tile as tile
    import concourse.mybir as mybir
    from concourse import bacc

    n_chunks = m_b // n_chunk
    n_groups = m_b // grp
    gpc = n_chunk // grp                 # groups per chunk
    assert m_b % n_chunk == 0 and n_chunk % grp == 0

    nc = bacc.Bacc("TRN2", target_bir_lowering=False, debug=False)
    atb = nc.dram_tensor(
        "ATB", [row_tiles, 128, 512], mybir.dt.float8e4, kind="ExternalInput"
    ).ap()
    btb = nc.dram_tensor(
        "BTB", [128, 4 * m_b], mybir.dt.float8e4, kind="ExternalInput"
    ).ap()
    nbg = nc.dram_tensor(
        "NBG", [128, n_groups], mybir.dt.float32, kind="ExternalInput"
    ).ap()
    mout = nc.dram_tensor(
        "M", [128, row_tiles], mybir.dt.float32, kind="ExternalOutput"
    ).ap()

    fp32 = mybir.dt.float32
    fp8 = mybir.dt.float8e4
    DR = mybir.MatmulPerfMode.DoubleRow
    add = mybir.AluOpType.add
    amin = mybir.AluOpType.min
    X = mybir.AxisListType.X

    # chunk groups -> one PSUM tile each; first group small so the DVE
    # drain of this row-tile starts early and finishes with the PE stream
    psgroups = []
    c = 0
    first = True
    while c < n_chunks:
        w = min(2 if first else 4, n_chunks - c)
        first = False
        psgroups.append((c, w))
        c += w

    with tile.TileContext(nc) as tc:
        with (
            tc.tile_pool(name="const", bufs=1) as cpool,
            tc.tile_pool(name="psum", bufs=2, space="PSUM") as pspool,
            tc.tile_pool(name="gm", bufs=row_tiles) as gmpool,
            tc.tile_pool(name="sfin", bufs=3) as spool,
            tc.tile_pool(name="mout", bufs=1) as mpool,
        ):
            # All of A^T resident: [128, row_tiles*512] bf16, one DMA per
            # row-tile on the sync queue (first matmul only needs piece 0).
            a_all = cpool.tile([128, row_tiles * 512], fp8)
            for it in range(row_tiles):
                nc.sync.dma_start(
                    out=a_all[:, it * 512 : (it + 1) * 512], in_=atb[it]
                )
            # B^T + nb on the scalar queue so they don't serialize behind ATB.
            # layout [p, kt(2), half(2), j(m_b)]
            bt_sb = cpool.tile([128, 4 * m_b], fp8)
            for c0, w in psgroups:
                for kt in range(2):
                    for half in range(2):
                        lo = kt * 2 * m_b + half * m_b + c0 * n_chunk
                        hi = lo + w * n_chunk
                        nc.scalar.dma_start(out=bt_sb[:, lo:hi], in_=btb[:, lo:hi])
            nbg_sb = cpool.tile([128, n_groups], fp32)
            nc.scalar.dma_start(out=nbg_sb[:], in_=nbg[:])
            m_sb = mpool.tile([128, row_tiles], fp32)

            gm_tiles = [gmpool.tile([128, n_groups], fp32, tag="gm", name=f"gm{i}") for i in range(row_tiles)]
            last_c0 = psgroups[-1][0]
            for c0, w in psgroups:
                for it in range(row_tiles):
                    ps = pspool.tile([128, 4 * n_chunk], fp32)
                    bt_v = bt_sb[:].rearrange("p (kt two j) -> p kt two j", kt=2, two=2)
                    for nl in range(w):
                        n = c0 + nl
                        for kt in range(2):
                            lhsT3 = a_all[
                                :, it * 512 + kt * 256 : it * 512 + (kt + 1) * 256
                            ].rearrange("p (two f) -> p two f", two=2)
                            nc.tensor.matmul(
                                ps[:, nl * n_chunk : (nl + 1) * n_chunk],
                                lhsT=lhsT3,
                                rhs=bt_v[:, kt, :, n * n_chunk : (n + 1) * n_chunk],
                                start=(kt == 0),
                                stop=(kt == 1),
                                perf_mode=DR,
                            )
                    nc.vector.tensor_reduce(
                        out=gm_tiles[it][:, c0 * gpc : (c0 + w) * gpc],
                        in_=ps[:, : w * n_chunk].rearrange("p (a b) -> p a b", b=grp),
                        axis=X,
                        op=amin,
                    )
                    if c0 == last_c0:
                        s_sb = spool.tile([128, n_groups], fp32)
                        nc.vector.tensor_tensor(
                            out=s_sb[:], in0=gm_tiles[it][:], in1=nbg_sb[:], op=add
                        )
                        nc.vector.tensor_reduce(
                            out=m_sb[:, it : it + 1], in_=s_sb[:], axis=X, op=amin
                        )
            nc.sync.dma_start(out=mout[:], in_=m_sb[:])
    nc.compile()
    return nc


def prep_inputs(A, B):
    """A: [N, 512] f32 (full), B: [M, 512] f32. Returns atb, btb, nbg."""
    e4 = ml_dtypes.float8_e4m3
    B32 = B.astype(np.float32)
    nb32 = (B32**2).sum(axis=1)
    # pad B with copies of column 0 (distance contributions duplicate, min unchanged)
    Bp = np.concatenate([B32, np.broadcast_to(B32[0:1], (M_PAD - M_B, D_FEAT))], axis=0)
    nbp = np.concatenate([nb32, np.broadcast_to(nb32[0:1], (M_PAD - M_B,))])
    order = np.argsort(nbp, kind="stable")
    Bs = Bp[order]
    nbs = nbp[order]

    # ATB: per-core row-tile blocks [core, 49, 128p(feat%128), 4k*128i] of -2A
    Apad = np.zeros((N_CORES, N_PAD, D_FEAT), np.float32)
    Apad[:, :N_PER_CORE, :] = (-2.0 * A.astype(np.float32)).reshape(
        N_CORES, N_PER_CORE, D_FEAT
    )
    # feature index = kt*256 + half*128 + p
    atb = np.ascontiguousarray(
        Apad.reshape(N_CORES, ROW_TILES, 128, 2, 2, 128).transpose(0, 1, 5, 3, 4, 2)
    ).reshape(N_CORES, ROW_TILES, 128, 512).astype(e4)

    # BTB: [128p, kt(2), half(2), 5120j] = Bs[j, kt*256+half*128+p]
    btb = np.ascontiguousarray(
        Bs.reshape(M_PAD, 2, 2, 128).transpose(3, 1, 2, 0)
    ).reshape(128, 4 * M_PAD).astype(e4)

    # per-group nb midpoint
    g = nbs.reshape(M_PAD // GRP, GRP)
    nb_mid = ((g.min(axis=1) + g.max(axis=1)) * 0.5).astype(np.float32)
    nbg = np.ascontiguousarray(
        np.broadcast_to(nb_mid[None, :], (128, M_PAD // GRP))
    ).astype(np.float32)
    return atb, btb, nbg


def _exact_rescore(A, B, cand):
    A64 = A[cand].astype(np.float64)
    B64 = B.astype(np.float64)
    na = (A64 * A64).sum(axis=1)[:, None]
    nb = (B64 * B64).sum(axis=1)[None, :]
    sq = na - 2.0 * (A64 @ B64.T) + nb
    d = np.sqrt(np.maximum(sq, 0.0))
    return d.min(axis=1)


def kernel(A, B, _trace=False):
    from concourse.bass_utils import run_bass_kernel_spmd

    global _compiled
    if _compiled is None:
        _compiled = build_program()
    nc = _compiled

    A = np.asarray(A, np.float32)
    B = np.asarray(B, np.float32)
    atb, btb, nbg = prep_inputs(A, B)
    in_maps = [{"ATB": atb[c], "BTB": btb, "NBG": nbg} for c in range(N_CORES)]
    res = run_bass_kernel_spmd(nc, in_maps, list(range(N_CORES)), trace=_trace)

    # Gather per-core m and undo the [128, 49] (p, it) layout -> row it*128+p
    m = np.concatenate(
        [res.results[c]["M"].T.reshape(-1)[:N_PER_CORE] for c in range(N_CORES)]
    )
    na = (A.astype(np.float64) ** 2).sum(axis=1)
    d_approx = np.sqrt(np.maximum(na + m, 0.0))
    v = d_approx.max()
    cand = np.where(d_approx >= v - DELTA)[0]
    d_exact = _exact_rescore(A, B, cand)
    w = int(np.argmax(d_exact))
    idx = int(cand[w])
    val = float(d_exact[w])
    out = (np.array(idx, dtype=np.int32), np.array(val, dtype=np.float32))
    if _trace:
        return out, res
    return out


# revision 10
# speedup vs baseline: 1.8772x; 1.0302x over previous
"""K-center style kernel: argmax_i min_j ||A_i - B_j|| on 8 NeuronCores.

Strategy:
  - Shard A row-wise over 8 cores (6250 rows each, padded to 6272 = 49*128).
  - Host: pad B to 5120 columns (copies of one real column), sort by
    nb = ||b||^2; group into runs of G=16 sorted columns with per-group
    nb midpoint.
  - Device (per core): matmuls (bf16, fp32 PSUM) produce -2 a_i.b_j in
    4-bank PSUM tiles (chunks of 512, bank aligned); one grouped DVE
    tensor_reduce(min) per PSUM tile gives per-group minima of -2p;
    per row-tile a tiny add(nb_mid) + min-reduce yields
    m[i] ~= min_j (nb_j - 2 a_i.b_j).
  - Host: D_approx = sqrt(max(na + m, 0)); select candidate rows within
    DELTA of the max; rescore candidates exactly in float64; return
    (argmax int32, max float32).

The host rescore makes the final answer exact regardless of device
precision; the device pass only needs the true argmax inside the
candidate set. Device error sources: bf16 input rounding (|D err| ~1e-2)
+ nb grouping (~1e-2). DELTA = 0.1 is far above both.
"""

import numpy as np
import ml_dtypes

N_CORES = 8
N_TOTAL = 50000
M_B = 5000
M_PAD = 5120                              # padded B columns (10 * 512)
D_FEAT = 512
N_PER_CORE = N_TOTAL // N_CORES          # 6250
ROW_TILES = 49                            # ceil(6250/128)
N_PAD = ROW_TILES * 128                   # 6272
K_TILES = 2                               # 512 / 256 (DoubleRow: 256 K per pass)
N_CHUNK = 512                             # matmul free dim = one fp32 PSUM bank
GRP = 64                                  # B columns per min-group (sorted by nb)

DELTA = 1.0  # candidate slack in D units (covers fp8 e4m3 + grouping error)

_compiled = None


def build_program(row_tiles=ROW_TILES, m_b=M_PAD, k_tiles=K_TILES, n_chunk=N_CHUNK, grp=GRP):
    import concourse.tile as tile
    import concourse.mybir as mybir
    from concourse import bacc

    n_chunks = m_b // n_chunk
    n_groups = m_b // grp
    gpc = n_chunk // grp                 # groups per chunk
    assert m_b % n_chunk == 0 and n_chunk % grp == 0

    nc = bacc.Bacc("TRN2", target_bir_lowering=False, debug=False)
    atb = nc.dram_tensor(
        "ATB", [row_tiles, 128, 512], mybir.dt.float8e4, kind="ExternalInput"
    ).ap()
    btb = nc.dram_tensor(
        "BTB", [128, 4 * m_b], mybir.dt.float8e4, kind="ExternalInput"
    ).ap()
    nbg = nc.dram_tensor(
        "NBG", [128, n_groups], mybir.dt.float32, kind="ExternalInput"
    ).ap()
    mout = nc.dram_tensor(
        "M", [128, row_tiles], mybir.dt.float32, kind="ExternalOutput"
    ).ap()

    fp32 = mybir.dt.float32
    fp8 = mybir.dt.float8e4
    DR = mybir.MatmulPerfMode.DoubleRow
    add = mybir.AluOpType.add
    amin = mybir.AluOpType.min
    X = mybir.AxisListType.X

    # chunk groups -> one PSUM tile each; first group small so the DVE
    # drain of this row-tile starts early and finishes with the PE stream
    psgroups = []
    c = 0
    first = True
    while c < n_chunks:
        w = min(2 if first else 4, n_chunks - c)
        first = False
        psgroups.append((c, w))
        c += w

    with tile.TileContext(nc) as tc:
        with (
            tc.tile_pool(name="const", bufs=1) as cpool,
            tc.tile_pool(name="psum", bufs=2, space="PSUM") as pspool,
            tc.tile_pool(name="gm", bufs=row_tiles) as gmpool,
            tc.tile_pool(name="sfin", bufs=3) as spool,
            tc.tile_pool(name="mout", bufs=1) as mpool,
        ):
            # All of A^T resident: [128, row_tiles*512] bf16, one DMA per
            # row-tile on the sync queue (first matmul only needs piece 0).
            # DMA order tuned for startup: A row-tile 0, then the first
            # chunk-group of B^T split across both HWDGE queues, then the
            # rest of A on sync and the rest of B^T on scalar.
            a_all = cpool.tile([128, row_tiles * 512], fp8)
            bt_sb = cpool.tile([128, 4 * m_b], fp8)  # [p, kt(2), half(2), j]
            nc.sync.dma_start(out=a_all[:, 0:512], in_=atb[0])
            c0, w = psgroups[0]
            for kt in range(2):
                for half in range(2):
                    lo = kt * 2 * m_b + half * m_b + c0 * n_chunk
                    hi = lo + w * n_chunk
                    eng = nc.sync if half == 0 else nc.scalar
                    eng.dma_start(out=bt_sb[:, lo:hi], in_=btb[:, lo:hi])
            for it in range(1, row_tiles):
                nc.sync.dma_start(
                    out=a_all[:, it * 512 : (it + 1) * 512], in_=atb[it]
                )
            for c0, w in psgroups[1:]:
                for kt in range(2):
                    for half in range(2):
                        lo = kt * 2 * m_b + half * m_b + c0 * n_chunk
                        hi = lo + w * n_chunk
                        nc.scalar.dma_start(out=bt_sb[:, lo:hi], in_=btb[:, lo:hi])
            nbg_sb = cpool.tile([128, n_groups], fp32)
            nc.scalar.dma_start(out=nbg_sb[:], in_=nbg[:])
            m_sb = mpool.tile([128, row_tiles], fp32)

            gm_tiles = [gmpool.tile([128, n_groups], fp32, tag="gm", name=f"gm{i}") for i in range(row_tiles)]
            last_c0 = psgroups[-1][0]
            for c0, w in psgroups:
                for it in range(row_tiles):
                    ps = pspool.tile([128, 4 * n_chunk], fp32)
                    bt_v = bt_sb[:].rearrange("p (kt two j) -> p kt two j", kt=2, two=2)
                    for nl in range(w):
                        n = c0 + nl
                        for kt in range(2):
                            lhsT3 = a_all[
                                :, it * 512 + kt * 256 : it * 512 + (kt + 1) * 256
                            ].rearrange("p (two f) -> p two f", two=2)
                            nc.tensor.matmul(
                                ps[:, nl * n_chunk : (nl + 1) * n_chunk],
                                lhsT=lhsT3,
                                rhs=bt_v[:, kt, :, n * n_chunk : (n + 1) * n_chunk],
                                start=(kt == 0),
                                stop=(kt == 1),
                                perf_mode=DR,
                            )
                    nc.vector.tensor_reduce(
                        out=gm_tiles[it][:, c0 * gpc : (c0 + w) * gpc],
                        in_=ps[:, : w * n_chunk].rearrange("p (a b) -> p a b", b=grp),
                        axis=X,
                        op=amin,
                    )
                    if c0 == last_c0:
                        s_sb = spool.tile([128, n_groups], fp32)
                        nc.vector.tensor_tensor(
                            out=s_sb[:], in0=gm_tiles[it][:], in1=nbg_sb[:], op=add
                        )
                        nc.vector.tensor_reduce(
                            out=m_sb[:, it : it + 1], in_=s_sb[:], axis=X, op=amin
                        )
            nc.sync.dma_start(out=mout[:], in_=m_sb[:])
    nc.compile()
    return nc


def prep_inputs(A, B):
    """A: [N, 512] f32 (full), B: [M, 512] f32. Returns atb, btb, nbg."""
    e4 = ml_dtypes.float8_e4m3
    B32 = B.astype(np.float32)
    nb32 = (B32**2).sum(axis=1)
    # pad B with copies of column 0 (distance contributions duplicate, min unchanged)
    Bp = np.concatenate([B32, np.broadcast_to(B32[0:1], (M_PAD - M_B, D_FEAT))], axis=0)
    nbp = np.concatenate([nb32, np.broadcast_to(nb32[0:1], (M_PAD - M_B,))])
    order = np.argsort(nbp, kind="stable")
    Bs = Bp[order]
    nbs = nbp[order]

    # ATB: per-core row-tile blocks [core, 49, 128p(feat%128), 4k*128i] of -2A
    Apad = np.zeros((N_CORES, N_PAD, D_FEAT), np.float32)
    Apad[:, :N_PER_CORE, :] = (-2.0 * A.astype(np.float32)).reshape(
        N_CORES, N_PER_CORE, D_FEAT
    )
    # feature index = kt*256 + half*128 + p
    atb = np.ascontiguousarray(
        Apad.reshape(N_CORES, ROW_TILES, 128, 2, 2, 128).transpose(0, 1, 5, 3, 4, 2)
    ).reshape(N_CORES, ROW_TILES, 128, 512).astype(e4)

    # BTB: [128p, kt(2), half(2), 5120j] = Bs[j, kt*256+half*128+p]
    btb = np.ascontiguousarray(
        Bs.reshape(M_PAD, 2, 2, 128).transpose(3, 1, 2, 0)
    ).reshape(128, 4 * M_PAD).astype(e4)

    # per-group nb midpoint
    g = nbs.reshape(M_PAD // GRP, GRP)
    nb_mid = ((g.min(axis=1) + g.max(axis=1)) * 0.5).astype(np.float32)
    nbg = np.ascontiguousarray(
        np.broadcast_to(nb_mid[None, :], (128, M_PAD // GRP))
    ).astype(np.float32)
    return atb, btb, nbg


def _exact_rescore(A, B, cand):
    A64 = A[cand].astype(np.float64)
    B64 = B.astype(np.float64)
    na = (A64 * A64).sum(axis=1)[:, None]
    nb = (B64 * B64).sum(axis=1)[None, :]
    sq = na - 2.0 * (A64 @ B64.T) + nb
    d = np.sqrt(np.maximum(sq, 0.0))
    return d.min(axis=1)


def kernel(A, B, _trace=False):
    from concourse.bass_utils import run_bass_kernel_spmd

    global _compiled
    if _compiled is None:
        _compiled = build_program()
    nc = _compiled

    A = np.asarray(A, np.float32)
    B = np.asarray(B, np.float32)
    atb, btb, nbg = prep_inputs(A, B)
    in_maps = [{"ATB": atb[c], "BTB": btb, "NBG": nbg} for c in range(N_CORES)]
    res = run_bass_kernel_spmd(nc, in_maps, list(range(N_CORES)), trace=_trace)

    # Gather per-core m and undo the [128, 49] (p, it) layout -> row it*128+p
    m = np.concatenate(
        [res.results[c]["M"].T.reshape(-1)[:N_PER_CORE] for c in range(N_CORES)]
    )
    na = (A.astype(np.float64) ** 2).sum(axis=1)
    d_approx = np.sqrt(np.maximum(na + m, 0.0))
    v = d_approx.max()
    cand = np.where(d_approx >= v - DELTA)[0]
    d_exact = _exact_rescore(A, B, cand)
    w = int(np.argmax(d_exact))
    idx = int(cand[w])
    val = float(d_exact[w])
    out = (np.array(idx, dtype=np.int32), np.array(val, dtype=np.float32))
    if _trace:
        return out, res
    return out
